# revision 7
# baseline (speedup 1.0000x reference)
"""Trainium2 Bass kernel for nn_CorrectTransformerAdaptor.

Strategy:
- Data-parallel over batch: 8 NeuronCores, one batch element each. No
  collectives; weights replicated.
- All activations/weights enter matmuls as bf16 (full PE speed, 213ns per
  128x128x512 matmul); PSUM accumulation and the residual stream are fp32.
- Activations live in "transposed" layout [feature, token] so every matmul
  chains without transposes:
    proj:    out[dout, t]  = lhsT(W.T chunk)[din, dout] x rhs(act)[din, t]
    scores:  ST[tk, tq]    = lhsT(k_h)[dk, tk] x rhs(q_h)[dk, tq]
    softmax: exp on ACT; denominators via ones-matmul (partition reduction)
    attnV:   OT[dv, tq]    = lhsT(vT)[tk, dv] x rhs(ET)[tk, tq]
- LayerNorm affine (g, b) is folded into the following projection weights on
  the host; V bias is folded into the attn-out bias (softmax sums to 1).
- Host pre-transposes/pre-tiles all weights so every DMA is contiguous.
"""

import numpy as np
import ml_dtypes

B, S, D_ENC = 8, 2048, 512
T, D, DFF, H, DK, FH, L = 1024, 1024, 2048, 8, 128, 256, 2
P = 128
EPS = 1e-12
NCORES = 8

_NC_CACHE = {}


def _build_bass():
    from contextlib import ExitStack
    import concourse.bass as bass
    import concourse.tile as tile
    import concourse.mybir as mybir
    from concourse import bacc

    f32 = mybir.dt.float32
    bf16 = mybir.dt.bfloat16
    AL = mybir.AluOpType
    AF = mybir.ActivationFunctionType
    ts = bass.ts

    nc = bacc.Bacc("TRN2", target_bir_lowering=False, debug=False)

    xt_d = nc.dram_tensor("xt", [8, P, T], bf16, kind="ExternalInput").ap()
    w1_d = nc.dram_tensor("w1", [16, P, 8, P], bf16, kind="ExternalInput").ap()
    b1_d = nc.dram_tensor("b1c", [P, 16], f32, kind="ExternalInput").ap()
    w2_d = nc.dram_tensor("w2", [P, 16, D], bf16, kind="ExternalInput").ap()
    b2_d = nc.dram_tensor("b2c", [P, 8], f32, kind="ExternalInput").ap()
    wq_d = nc.dram_tensor("wq", [L, 8, P, 8, P], bf16, kind="ExternalInput").ap()
    wk_d = nc.dram_tensor("wk", [L, 8, P, 8, P], bf16, kind="ExternalInput").ap()
    wv_d = nc.dram_tensor("wv", [L, 2, P, 8, 512], bf16, kind="ExternalInput").ap()
    wo_d = nc.dram_tensor("wo", [L, 8, P, 8, P], bf16, kind="ExternalInput").ap()
    bq_d = nc.dram_tensor("bqc", [L, P, 8], f32, kind="ExternalInput").ap()
    bk_d = nc.dram_tensor("bkc", [L, P, 8], f32, kind="ExternalInput").ap()
    bo_d = nc.dram_tensor("boc", [L, P, 8], f32, kind="ExternalInput").ap()
    fw1_d = nc.dram_tensor("fw1", [L, 2, P, 8, P], bf16, kind="ExternalInput").ap()
    fb1_d = nc.dram_tensor("fb1c", [L, P, 2], f32, kind="ExternalInput").ap()
    fw2_d = nc.dram_tensor("fw2", [L, P, 2, D], bf16, kind="ExternalInput").ap()
    fb2_d = nc.dram_tensor("fb2c", [L, P, 8], f32, kind="ExternalInput").ap()
    ones_d = nc.dram_tensor("ones", [P, P], bf16, kind="ExternalInput").ap()
    out_d = nc.dram_tensor("out", [8, P, T], f32, kind="ExternalOutput").ap()

    es = ExitStack()
    with tile.TileContext(nc) as tc, es:
        consts = es.enter_context(tc.tile_pool(name="consts", bufs=1))
        wc128 = es.enter_context(tc.tile_pool(name="wc128", bufs=4))
        pp = es.enter_context(tc.tile_pool(name="pp", bufs=8, space="PSUM"))

        ones = consts.tile([P, P], bf16)
        nc.sync.dma_start(ones[:], ones_d)
        eps_t = consts.tile([P, 1], f32)
        nc.vector.memset(eps_t[:], EPS)
        b1c = consts.tile([P, 16], f32)
        nc.sync.dma_start(b1c[:], b1_d)
        b2c = consts.tile([P, 8], f32)
        nc.sync.dma_start(b2c[:], b2_d)
        bqc = consts.tile([P, L, 8], f32)
        bkc = consts.tile([P, L, 8], f32)
        boc = consts.tile([P, L, 8], f32)
        fb1c = consts.tile([P, L, 2], f32)
        fb2c = consts.tile([P, L, 8], f32)
        for l in range(L):
            nc.sync.dma_start(bqc[:, l, :], bq_d[l])
            nc.sync.dma_start(bkc[:, l, :], bk_d[l])
            nc.sync.dma_start(boc[:, l, :], bo_d[l])
            nc.sync.dma_start(fb1c[:, l, :], fb1_d[l])
            nc.sync.dma_start(fb2c[:, l, :], fb2_d[l])

        # ---------------- downsample MLP ----------------
        respool = es.enter_context(tc.tile_pool(name="resp", bufs=1))
        resid = respool.tile([P, 8, T], f32)
        with tc.tile_pool(name="dsp", bufs=1) as dsp:
            xt_s = dsp.tile([P, 8, T], bf16, tag="xt")
            for k in range(8):
                nc.sync.dma_start(xt_s[:, k, :], xt_d[k])
            h1 = dsp.tile([P, 16, T], bf16, tag="h1")
            w2s = dsp.tile([P, 16, D], bf16, tag="w2s")
            nc.sync.dma_start(w2s[:], w2_d)

            for ff in range(16):
                w1c = wc128.tile([P, 8, P], bf16, tag="wc")
                nc.sync.dma_start(w1c[:], w1_d[ff])
                for t in range(2):
                    ps = pp.tile([P, 512], f32, tag="ps")
                    for k in range(8):
                        nc.tensor.matmul(ps[:], w1c[:, k, :], xt_s[:, k, ts(t, 512)],
                                         start=(k == 0), stop=(k == 7))
                    nc.scalar.activation(h1[:, ff, ts(t, 512)], ps[:], AF.Relu,
                                         bias=b1c[:, ff:ff + 1])

            for t in range(2):
                pss = [pp.tile([P, 512], f32, tag="ps", name=f"ds2_{t}_{dl}")
                       for dl in range(8)]
                for k in range(16):
                    for dl in range(8):
                        nc.tensor.matmul(pss[dl][:], w2s[:, k, ts(dl, P)],
                                         h1[:, k, ts(t, 512)],
                                         start=(k == 0), stop=(k == 15))
                for dl in range(8):
                    nc.scalar.activation(resid[:, dl, ts(t, 512)], pss[dl][:],
                                         AF.Identity, bias=b2c[:, dl:dl + 1])

        big = es.enter_context(tc.tile_pool(name="big", bufs=4))

        def layernorm(tag):
            """resid (fp32) -> normalized bf16 tile from `big` (affine folded
            into the downstream weights on the host)."""
            with tc.tile_pool(name=f"ln_{tag}", bufs=2) as lnp, \
                 tc.tile_pool(name=f"lns_{tag}", bufs=1) as lns:
                dst = big.tile([P, 8, T], bf16, tag="big", name=f"xh_{tag}")
                s1 = [pp.tile([P, 512], f32, tag="ps", name=f"s1_{tag}{t}")
                      for t in range(2)]
                s2 = [pp.tile([P, 512], f32, tag="ps", name=f"s2_{tag}{t}")
                      for t in range(2)]
                for k in range(8):
                    rc = lnp.tile([P, T], bf16, tag="rc", name=f"rc_{tag}{k}")
                    nc.vector.tensor_copy(rc[:], resid[:, k, :])
                    sq = lnp.tile([P, T], bf16, tag="sq", name=f"sq_{tag}{k}")
                    nc.scalar.square(sq[:], rc[:])
                    for t in range(2):
                        nc.tensor.matmul(s1[t][:], ones[:], rc[:, ts(t, 512)],
                                         start=(k == 0), stop=(k == 7))
                        nc.tensor.matmul(s2[t][:], ones[:], sq[:, ts(t, 512)],
                                         start=(k == 0), stop=(k == 7))
                m_sb = lns.tile([P, T], f32, tag="m", name=f"m_{tag}")
                ms_sb = lns.tile([P, T], f32, tag="msb", name=f"ms_{tag}")
                s_sb = lns.tile([P, T], f32, tag="s", name=f"s_{tag}")
                for t in range(2):
                    tsl = ts(t, 512)
                    nc.vector.tensor_scalar_mul(m_sb[:, tsl], s1[t][:], 1.0 / D)
                    tmp = lnp.tile([P, 512], f32, tag="tmp", name=f"tmp_{tag}{t}")
                    nc.vector.tensor_mul(tmp[:], m_sb[:, tsl], m_sb[:, tsl])
                    nc.vector.scalar_tensor_tensor(
                        tmp[:], s2[t][:], 1.0 / D, tmp[:],
                        op0=AL.mult, op1=AL.subtract)
                    nc.scalar.activation(tmp[:], tmp[:], AF.Sqrt, bias=eps_t[:])
                    nc.vector.reciprocal(s_sb[:, tsl], tmp[:])
                # ms = m * s ; xhat = x*s - ms
                nc.vector.tensor_mul(ms_sb[:], m_sb[:], s_sb[:])
                for k in range(8):
                    nc.vector.tensor_mul(dst[:, k, :], resid[:, k, :], s_sb[:])
                    nc.vector.tensor_sub(dst[:, k, :], dst[:, k, :], ms_sb[:])
                return dst

        for l in range(L):
            xh = layernorm(f"l{l}a")

            # ---- V projection: vT[t, dv] (token partitions) ----
            vT = big.tile([P, 8, D], bf16, tag="big", name=f"vT{l}")
            with tc.tile_pool(name=f"wvp{l}", bufs=2) as wvp:
                for g in range(2):
                    wvc = wvp.tile([P, 8, 512], bf16, tag="wv", name=f"wv{l}{g}")
                    nc.sync.dma_start(wvc[:], wv_d[l, g])
                    for tt in range(8):
                        ps = pp.tile([P, 512], f32, tag="ps", name=f"psv{l}{g}{tt}")
                        for k in range(8):
                            nc.tensor.matmul(ps[:], xh[:, k, ts(tt, P)],
                                             wvc[:, k, :],
                                             start=(k == 0), stop=(k == 7))
                        nc.vector.tensor_copy(vT[:, tt, ts(g, 512)], ps[:])

            # ---- Q/K projections per head: [dk, t] layout ----
            q = big.tile([P, 8, T], bf16, tag="big", name=f"q{l}")
            kk_ = big.tile([P, 8, T], bf16, tag="big", name=f"k{l}")
            for h in range(H):
                wqc = wc128.tile([P, 8, P], bf16, tag="wc", name=f"wq{l}{h}")
                nc.sync.dma_start(wqc[:], wq_d[l, h])
                wkc = wc128.tile([P, 8, P], bf16, tag="wc", name=f"wk{l}{h}")
                nc.sync.dma_start(wkc[:], wk_d[l, h])
                for t in range(2):
                    tsl = ts(t, 512)
                    psq = pp.tile([P, 512], f32, tag="ps", name=f"psq{l}{h}{t}")
                    psk = pp.tile([P, 512], f32, tag="ps", name=f"psk{l}{h}{t}")
                    for k in range(8):
                        nc.tensor.matmul(psq[:], wqc[:, k, :], xh[:, k, tsl],
                                         start=(k == 0), stop=(k == 7))
                        nc.tensor.matmul(psk[:], wkc[:, k, :], xh[:, k, tsl],
                                         start=(k == 0), stop=(k == 7))
                    nc.scalar.activation(q[:, h, tsl], psq[:], AF.Identity,
                                         bias=bqc[:, l, h:h + 1])
                    nc.scalar.activation(kk_[:, h, tsl], psk[:], AF.Identity,
                                         bias=bkc[:, l, h:h + 1])

            # ---- attention ----
            OT = big.tile([P, 8, T], bf16, tag="big", name=f"OT{l}")
            with tc.tile_pool(name=f"att{l}", bufs=1) as att:
                for h in range(H):
                    ssum = [pp.tile([P, 512], f32, tag="ps", name=f"ssum{l}{h}{t}")
                            for t in range(2)]
                    sot = [pp.tile([P, 512], f32, tag="ps", name=f"sot{l}{h}{t}")
                           for t in range(2)]
                    for tk in range(8):
                        et = att.tile([P, T], bf16, tag="et", name=f"et{l}{h}{tk}")
                        for t in range(2):
                            st = pp.tile([P, 512], f32, tag="ps",
                                         name=f"st{l}{h}{tk}{t}")
                            nc.tensor.matmul(st[:], kk_[:, h, ts(tk, P)],
                                             q[:, h, ts(t, 512)],
                                             start=True, stop=True)
                            nc.scalar.activation(et[:, ts(t, 512)], st[:],
                                                 AF.Exp, scale=float(DK) ** -0.5)
                        for t in range(2):
                            nc.tensor.matmul(ssum[t][:], ones[:],
                                             et[:, ts(t, 512)],
                                             start=(tk == 0), stop=(tk == 7))
                            nc.tensor.matmul(sot[t][:], vT[:, tk, ts(h, P)],
                                             et[:, ts(t, 512)],
                                             start=(tk == 0), stop=(tk == 7))
                    iv = att.tile([P, T], f32, tag="iv", name=f"iv{l}{h}")
                    for t in range(2):
                        nc.vector.reciprocal(iv[:, ts(t, 512)], ssum[t][:])
                        nc.vector.tensor_mul(OT[:, h, ts(t, 512)], sot[t][:],
                                             iv[:, ts(t, 512)])

            # ---- attn out projection + residual ----
            for do in range(8):
                woc = wc128.tile([P, 8, P], bf16, tag="wc", name=f"wo{l}{do}")
                nc.sync.dma_start(woc[:], wo_d[l, do])
                for t in range(2):
                    tsl = ts(t, 512)
                    ps = pp.tile([P, 512], f32, tag="ps", name=f"pso{l}{do}{t}")
                    for k in range(8):
                        nc.tensor.matmul(ps[:], woc[:, k, :], OT[:, k, tsl],
                                         start=(k == 0), stop=(k == 7))
                    nc.vector.scalar_tensor_tensor(
                        resid[:, do, tsl], ps[:], boc[:, l, do:do + 1],
                        resid[:, do, tsl], op0=AL.add, op1=AL.add)

            # ---- FFN ----
            xh2 = layernorm(f"l{l}b")
            with tc.tile_pool(name=f"ffn{l}", bufs=1) as ffn:
                hf = ffn.tile([P, 2, T], bf16, tag="hf", name=f"hf{l}")
                for ff in range(2):
                    fwc = wc128.tile([P, 8, P], bf16, tag="wc", name=f"fw{l}{ff}")
                    nc.sync.dma_start(fwc[:], fw1_d[l, ff])
                    for t in range(2):
                        ps = pp.tile([P, 512], f32, tag="ps", name=f"psf{l}{ff}{t}")
                        for k in range(8):
                            nc.tensor.matmul(ps[:], fwc[:, k, :], xh2[:, k, ts(t, 512)],
                                             start=(k == 0), stop=(k == 7))
                        nc.scalar.activation(hf[:, ff, ts(t, 512)], ps[:], AF.Relu,
                                             bias=fb1c[:, l, ff:ff + 1])
                fw2s = ffn.tile([P, 2, D], bf16, tag="fw2", name=f"fw2{l}")
                nc.sync.dma_start(fw2s[:], fw2_d[l])
                for do in range(8):
                    for t in range(2):
                        tsl = ts(t, 512)
                        ps = pp.tile([P, 512], f32, tag="ps", name=f"psg{l}{do}{t}")
                        for k in range(2):
                            nc.tensor.matmul(ps[:], fw2s[:, k, ts(do, P)],
                                             hf[:, k, tsl],
                                             start=(k == 0), stop=(k == 1))
                        nc.vector.scalar_tensor_tensor(
                            resid[:, do, tsl], ps[:], fb2c[:, l, do:do + 1],
                            resid[:, do, tsl], op0=AL.add, op1=AL.add)

        for k in range(8):
            nc.sync.dma_start(out_d[k], resid[:, k, :])

    nc.compile()
    return nc


def _col(v, nb):
    """bias vector (nb*128,) -> [128, nb] column layout (partition-major)."""
    return np.ascontiguousarray(v.reshape(nb, P).T, dtype=np.float32)


def _prep_weights(W1, b1, W2, b2, ln1_g, ln1_b, ln2_g, ln2_b,
                  Wq, bq, Wk, bk, Wv, bv, Wo, bo, Fw1, Fb1, Fw2, Fb2):
    bf = ml_dtypes.bfloat16
    d = {}
    W1T = W1.T.astype(np.float32)                       # [1024, 2048]
    d["w1"] = np.ascontiguousarray(
        W1T.reshape(8, P, 16, P).transpose(2, 1, 0, 3)).astype(bf)
    d["b1c"] = _col(b1, 16)
    W2T = W2.T.astype(np.float32)                       # [2048, 1024]
    d["w2"] = np.ascontiguousarray(
        W2T.reshape(16, P, D).transpose(1, 0, 2)).astype(bf)
    d["b2c"] = _col(b2, 8)

    wq_l, wk_l, wv_l, wo_l = [], [], [], []
    bq_l, bk_l, bo_l = [], [], []
    fw1_l, fb1_l, fw2_l, fb2_l = [], [], [], []
    for l in range(L):
        g1, be1 = ln1_g[l].astype(np.float64), ln1_b[l].astype(np.float64)
        g2, be2 = ln2_g[l].astype(np.float64), ln2_b[l].astype(np.float64)
        WqT = (g1[:, None] * Wq[l].T.astype(np.float64))
        WkT = (g1[:, None] * Wk[l].T.astype(np.float64))
        WvT = (g1[:, None] * Wv[l].T.astype(np.float64))
        bq_f = bq[l].astype(np.float64) + Wq[l].astype(np.float64) @ be1
        bk_f = bk[l].astype(np.float64) + Wk[l].astype(np.float64) @ be1
        bv_f = bv[l].astype(np.float64) + Wv[l].astype(np.float64) @ be1
        WoT = Wo[l].T.astype(np.float64)
        bo_f = bo[l].astype(np.float64) + Wo[l].astype(np.float64) @ bv_f
        Fw1T = (g2[:, None] * Fw1[l].T.astype(np.float64))
        fb1_f = Fb1[l].astype(np.float64) + Fw1[l].astype(np.float64) @ be2
        Fw2T = Fw2[l].T.astype(np.float64)

        wq_l.append(WqT.reshape(8, P, 8, P).transpose(2, 1, 0, 3))
        wk_l.append(WkT.reshape(8, P, 8, P).transpose(2, 1, 0, 3))
        wv_l.append(WvT.reshape(8, P, 2, 512).transpose(2, 1, 0, 3))
        wo_l.append(WoT.reshape(8, P, 8, P).transpose(2, 1, 0, 3))
        bq_l.append(_col(np.asarray(bq_f, np.float32), 8))
        bk_l.append(_col(np.asarray(bk_f, np.float32), 8))
        bo_l.append(_col(np.asarray(bo_f, np.float32), 8))
        fw1_l.append(Fw1T.reshape(8, P, 2, P).transpose(2, 1, 0, 3))
        fb1_l.append(_col(np.asarray(fb1_f, np.float32), 2))
        fw2_l.append(Fw2T.reshape(2, P, D).transpose(1, 0, 2))
        fb2_l.append(_col(Fb2[l], 8))

    d["wq"] = np.ascontiguousarray(np.stack(wq_l)).astype(bf)
    d["wk"] = np.ascontiguousarray(np.stack(wk_l)).astype(bf)
    d["wv"] = np.ascontiguousarray(np.stack(wv_l)).astype(bf)
    d["wo"] = np.ascontiguousarray(np.stack(wo_l)).astype(bf)
    d["bqc"] = np.stack(bq_l)
    d["bkc"] = np.stack(bk_l)
    d["boc"] = np.stack(bo_l)
    d["fw1"] = np.ascontiguousarray(np.stack(fw1_l)).astype(bf)
    d["fb1c"] = np.stack(fb1_l)
    d["fw2"] = np.ascontiguousarray(np.stack(fw2_l)).astype(bf)
    d["fb2c"] = np.stack(fb2_l)
    d["ones"] = np.ones((P, P), dtype=bf)
    return d


def kernel(**inputs):
    from concourse import bass_utils

    if "nc" not in _NC_CACHE:
        _NC_CACHE["nc"] = _build_bass()
    nc = _NC_CACHE["nc"]

    x = np.asarray(inputs["x"], dtype=np.float32)
    wd = _prep_weights(**{k: np.asarray(v) for k, v in inputs.items() if k != "x"})

    bf = ml_dtypes.bfloat16
    in_maps = []
    for b in range(NCORES):
        xt = np.ascontiguousarray(
            x[b].reshape(T, D).T.reshape(8, P, T)).astype(bf)
        m = dict(wd)
        m["xt"] = xt
        in_maps.append(m)

    res = bass_utils.run_bass_kernel_spmd(nc, in_maps, core_ids=list(range(NCORES)))
    outs = []
    for b in range(NCORES):
        o = res.results[b]["out"]                    # [8, 128, 1024] = [D, T]
        outs.append(o.reshape(D, T).T)
    return np.ascontiguousarray(np.stack(outs), dtype=np.float32)
